# revision 2
# baseline (speedup 1.0000x reference)
"""Trainium2 Bass kernel for nn_CTSimGLM: GLM spike-train simulation.

Key algorithmic facts exploited:
  - All 32 repeats of the reference are bit-identical (deterministic sigmoid
    rates) -> compute each batch once, broadcast over repeats on the host.
  - With repeats collapsed there are only 4 independent lanes (batches), so
    pure batch-parallelism fits the 8 cores with NO cross-core communication:
    each core streams the FULL stimulus of one batch (16.4 MB fp16) plus all
    24 coupling channels and computes its gensig entirely locally. This
    removes the collective (15us constant + staging) that dominated the
    pixel-sharded design.
  - The Jacobi fixed-point iteration on the spike raster converges ~2.5x per
    sweep; 5 fp16 sweeps reach ~9e-3 max error (tolerance 2e-2).

Per core, uniform SPMD program (core c simulates batch c%4; cores 4-7 are
redundant duplicates):
  1. stim fp16 streamed as 64 half-chunk DMAs, time-major (all t<1000 halves
     first), balanced over the 3 DMA queues (SP/ACT/Pool); 512 matvec
     matmuls produce a time-major spat raster [128 t-local, 16 chunks].
  2. gensig raster [128, 14] = coupling conv (24 channels, Toeplitz matmuls,
     runs early) + bias + timecourse conv (3 Toeplitz matmuls after the last
     stim byte), accumulated in one PSUM bank, cast fp16.
  3. 5 fp16 Jacobi sweeps, each: 1 identity matmul (G add) + 3 Toeplitz
     feedback matmuls into PSUM + 1 sigmoid ACT -> X raster [128, 16]
     (chunk c row i is t = 128c - 6 + i; cols 0:2 hold the initial window).
  4. single [128, 14] fp32 DMA out per core; the host takes core b's raster
     for batch b, prepends the initial window, broadcasts over repeats.
"""

import os
import time
from contextlib import ExitStack

import numpy as np

import concourse.bass as bass
import concourse.bacc as bacc
import concourse.tile as tile
import concourse.mybir as mybir
from concourse.bass_utils import run_bass_kernel_spmd

ts = bass.ts

B, P, T, K, C, R = 4, 4096, 2000, 250, 24, 32
NCORES = 8
PCH = P // 128               # 32 pixel chunks (full image per core)
NCH = 16                     # X chunks; col c, row i <-> t = 128c - 6 + i
NJ = 14                      # out chunks 2..15
NSW = 5                      # fp16 Jacobi sweeps

F32 = mybir.dt.float32
F16 = mybir.dt.float16
SIG = mybir.ActivationFunctionType.Sigmoid


def _toeplitz(filt, shift):
    """3 stacked [128,128] tiles: F_d[i, jj] = filt[128*d + shift + i - jj]."""
    i = np.arange(128)[:, None]
    jj = np.arange(128)[None, :]
    out = np.zeros((3, 128, 128), np.float32)
    for d in range(3):
        idx = 128 * d + shift + i - jj
        valid = (idx >= 0) & (idx < K)
        out[d] = np.where(valid, filt[np.clip(idx, 0, K - 1)], 0.0)
    return out


def _build_nc():
    nc = bacc.Bacc(
        "TRN2",
        target_bir_lowering=False,
        debug=False,
        num_devices=NCORES,
    )

    stim_d = nc.dram_tensor("stim_b", [PCH, 128, T], F16, kind="ExternalInput")
    # packA: sf[0:32] cspk[32:416] x0[416:418] ones-row[418:546]
    #        bias-row[546:560] (rows 1.. of the row-vector cols are zero)
    packA_d = nc.dram_tensor("packA", [128, 560], F16, kind="ExternalInput")
    # coupT: 24 ch x 3 d x [128, 128]
    coupT_d = nc.dram_tensor("coupT", [128, C * 3 * 128], F16, kind="ExternalInput")
    # packT: tcT[0:384];  packF: identh[0:128] fbT[128:512]
    packT_d = nc.dram_tensor("packT", [128, 384], F16, kind="ExternalInput")
    packF_d = nc.dram_tensor("packF", [128, 512], F16, kind="ExternalInput")
    out_d = nc.dram_tensor("out_x", [128, NJ], F32, kind="ExternalOutput")

    with tile.TileContext(nc) as tc, ExitStack() as ctx:
        consts = ctx.enter_context(tc.tile_pool(name="consts", bufs=1))

        packA_s = consts.tile([128, 560], F16)
        nc.gpsimd.dma_start(packA_s[:], packA_d[:])
        coupT_s = consts.tile([128, C * 3 * 128], F16)
        packT_s = consts.tile([128, 384], F16)
        packF_s = consts.tile([128, 512], F16)
        sf_s = packA_s[:, 0:PCH]
        cspk_s = packA_s[:, PCH : PCH + C * NCH]
        x0_s = packA_s[:, 416:418]
        ones_s = packA_s[0:1, 418:546]
        bias_s = packA_s[0:1, 546:560]
        tcT_s = packT_s[:, 0:384]
        identh_s = packF_s[:, 0:128]
        fbT_s = packF_s[:, 128:512]

        with (
            tc.tile_pool(name="stim", bufs=PCH) as stim_pool,
            tc.tile_pool(name="psum_sp", bufs=4, space="PSUM") as psum_sp,
            tc.tile_pool(name="psum_g", bufs=1, space="PSUM") as psum_g,
            tc.tile_pool(name="psum_xr", bufs=2, space="PSUM") as psum_xr,
        ):
            spat_t = consts.tile([128, NCH], F16)
            nc.vector.memset(spat_t[:], 0.0)

            # 64 half-chunk DMAs, time-major; coupT rides between the phases
            # split 3 ways so every queue carries a share
            ENG = [nc.sync, nc.scalar, nc.gpsimd]
            sts = {}
            for pc in range(PCH):
                sts[pc] = stim_pool.tile([128, T], F16, tag="st", name=f"st{pc}")
            for pc in range(PCH):
                ENG[pc % 3].dma_start(sts[pc][:, 0:1000], stim_d[pc, :, 0:1000])
            nc.gpsimd.dma_start(packT_s[:], packT_d[:])
            third = C * 3 * 128 // 3
            for q in range(3):
                ENG[q].dma_start(
                    coupT_s[:, q * third : (q + 1) * third],
                    coupT_d[:, q * third : (q + 1) * third],
                )
            for pc in range(PCH):
                ENG[(pc + 1) % 3].dma_start(
                    sts[pc][:, 1000:2000], stim_d[pc, :, 1000:2000]
                )
            nc.gpsimd.dma_start(packF_s[:], packF_d[:])

            # spatial projection: accumulate 32 pixel chunks per time chunk
            for ttg in range(4):
                pst = psum_sp.tile([128, 4], F32, tag="sp")
                for tt in range(4):
                    c = ttg * 4 + tt
                    hh = 128 if c < NCH - 1 else 80
                    for pc in range(PCH):
                        nc.tensor.matmul(
                            pst[0:hh, tt : tt + 1],
                            lhsT=sts[pc][:, 128 * c : 128 * c + hh],
                            rhs=sf_s[:, pc : pc + 1],
                            start=(pc == 0),
                            stop=(pc == PCH - 1),
                        )
                if ttg < 3:
                    nc.vector.tensor_copy(spat_t[:, ts(ttg, 4)], pst[:, 0:4])
                else:
                    nc.vector.tensor_copy(spat_t[:, 12:15], pst[:, 0:3])
                    nc.vector.tensor_copy(spat_t[0:80, 15:16], pst[0:80, 3:4])

            # X rasters (cols 0:2 = initial spike window)
            xah = consts.tile([128, NCH], F16)
            xbh = consts.tile([128, NCH], F16)
            xf32 = consts.tile([128, NCH], F32)
            nc.vector.memset(xah[:], 0.0)
            nc.vector.memset(xbh[:], 0.0)
            nc.vector.tensor_copy(xah[:, 0:2], x0_s)
            nc.vector.tensor_copy(xbh[:, 0:2], x0_s)

            # gensig raster: coupling (early) + bias + timecourse (after spat)
            gh = consts.tile([128, NJ], F16)
            pg = psum_g.tile([128, NJ], F32, tag="pg")
            first = True
            for ch in range(C):
                for d in range(3):
                    nc.tensor.matmul(
                        pg[:],
                        lhsT=coupT_s[:, ts(ch * 3 + d, 128)],
                        rhs=cspk_s[:, ch * NCH + d : ch * NCH + d + NJ],
                        start=first,
                        stop=False,
                    )
                    first = False
            nc.tensor.matmul(
                pg[:], lhsT=ones_s, rhs=bias_s, start=False, stop=False
            )
            for d in range(3):
                nc.tensor.matmul(
                    pg[:],
                    lhsT=tcT_s[:, ts(d, 128)],
                    rhs=spat_t[:, d : d + NJ],
                    start=False,
                    stop=(d == 2),
                )
            nc.vector.tensor_copy(gh[:], pg[:])
            # sigmoid table load as soon as the raster exists
            sigwarm = consts.tile([1, 1], F32)
            nc.scalar.activation(sigwarm[:], gh[0:1, 0:1], SIG)

            # ---- Jacobi sweeps ----
            def sweep(src, dst_sl, first_sweep):
                # on sweep 1 X is zero beyond the initial window (cols 0:2):
                # the d=2 feedback matmul is a no-op and d=0/1 only reach the
                # first output chunks
                px = psum_xr.tile([128, NJ], F32, tag="px")
                nc.tensor.matmul(
                    px[:], lhsT=identh_s[:], rhs=gh[:], start=True, stop=False
                )
                nds = ((2, 1), (1, 1)) if first_sweep else ((NJ, 1), (NJ, 1), (NJ, 1))
                for d, (w, _) in enumerate(nds):
                    nc.tensor.matmul(
                        px[:, 0:w],
                        lhsT=fbT_s[:, ts(d, 128)],
                        rhs=src[:, d : d + w],
                        start=False,
                        stop=(d == len(nds) - 1),
                    )
                nc.scalar.activation(dst_sl, px[:], SIG)

            cur, nxt = xah, xbh
            for s in range(NSW - 1):
                sweep(cur, nxt[:, 2:NCH], s == 0)
                cur, nxt = nxt, cur
            sweep(cur, xf32[:, 2:NCH], False)

            nc.sync.dma_start(out_d[:], xf32[:, 2:NCH])

    nc.compile()
    return nc


_NC_CACHE = None


def _get_nc():
    global _NC_CACHE
    if _NC_CACHE is None:
        _NC_CACHE = _build_nc()
    return _NC_CACHE


def make_in_maps(
    stim_movie,
    initial_spike_section,
    coupled_cell_spikes,
    spatial_filter,
    timecourse_filter,
    feedback_filter,
    coupling_filters,
    bias,
):
    fbT = _toeplitz(feedback_filter, -6).astype(np.float16)
    tcT = _toeplitz(timecourse_filter, 0).astype(np.float16)
    identh = np.eye(128, dtype=np.float16)
    packT = np.concatenate([tcT[d] for d in range(3)], axis=1)
    packF = np.concatenate(
        [identh] + [fbT[d] for d in range(3)], axis=1
    )
    coupT = np.concatenate(
        [
            _toeplitz(coupling_filters[ch], 0)[d]
            for ch in range(C)
            for d in range(3)
        ],
        axis=1,
    ).astype(np.float16)

    onesrow = np.zeros((128, 128), np.float16)
    onesrow[0, :] = 1.0
    biasrow = np.zeros((128, NJ), np.float16)
    biasrow[0, :] = np.float16(bias[0])
    sf_h = spatial_filter.astype(np.float16).reshape(PCH, 128).T
    stim_h = stim_movie.astype(np.float16)

    in_maps = []
    for core in range(NCORES):
        b = core % B
        # initial window raster: x0[i, c] = init[b, 128c-6+i] for c=0,1
        x0 = np.zeros((128, 2), np.float16)
        for c in range(2):
            t = 128 * c - 6 + np.arange(128)
            valid = (t >= 0) & (t < K)
            x0[valid, c] = initial_spike_section[b, t[valid]].astype(np.float16)
        # coupled-spike raster [128, (ch, c)]
        cspk_r = np.zeros((128, C * NCH), np.float16)
        for ch in range(C):
            for c in range(NCH):
                t = 128 * c + np.arange(128)
                valid = t < T
                cspk_r[valid, ch * NCH + c] = coupled_cell_spikes[b, ch, t[valid]]
        packA = np.concatenate(
            [sf_h.astype(np.float16), cspk_r, x0, onesrow, biasrow], axis=1
        )
        in_maps.append(
            {
                "stim_b": np.ascontiguousarray(
                    stim_h[b].reshape(PCH, 128, T)
                ),
                "packA": np.ascontiguousarray(packA),
                "coupT": np.ascontiguousarray(coupT),
                "packT": np.ascontiguousarray(packT),
                "packF": np.ascontiguousarray(packF),
            }
        )
    return in_maps


def kernel(**inputs):
    assert int(inputs["n_repeats"]) == R
    in_maps = make_in_maps(
        np.asarray(inputs["stim_movie"], np.float32),
        np.asarray(inputs["initial_spike_section"], np.float32),
        np.asarray(inputs["coupled_cell_spikes"], np.float32),
        np.asarray(inputs["spatial_filter"], np.float32),
        np.asarray(inputs["timecourse_filter"], np.float32),
        np.asarray(inputs["feedback_filter"], np.float32),
        np.asarray(inputs["coupling_filters"], np.float32),
        np.asarray(inputs["bias"], np.float32),
    )
    nc = _get_nc()
    # the axon tunnel occasionally wedges (transient LoadExecutable failures);
    # a short backoff + core reset clears it
    last_exc = None
    for attempt in range(3):
        try:
            res = run_bass_kernel_spmd(
                nc,
                in_maps,
                core_ids=list(range(NCORES)),
                trace=bool(int(os.environ.get("KERNEL_TRACE", "0"))),
            )
            break
        except Exception as e:
            last_exc = e
            os.environ["NEURON_RT_RESET_CORES"] = "1"
            time.sleep(20 * (attempt + 1))
    else:
        raise last_exc
    out = np.empty((B, T), np.float32)
    out[:, :K] = np.asarray(inputs["initial_spike_section"], np.float32)
    for b in range(B):
        out_x = res.results[b]["out_x"]  # [128, 14], col j, row i; t = 250+128j+i
        out[b, K:] = out_x.T.reshape(NJ * 128)[: T - K]
    kernel.last_results = res
    return np.ascontiguousarray(
        np.broadcast_to(out[:, None, :], (B, R, T)).astype(np.float32)
    )
